# revision 19
# baseline (speedup 1.0000x reference)
"""Single-head causal attention with ALiBi (B=4, T=4096, C=HS=64) on 8 TRN2 cores.

Math: out = softmax(mask((x Wq)(x Wk)^T * C^-0.5 + (j-i)*slope)) @ (x Wv)

ALiBi slope 2^-0.5 makes the softmax an effective 256-wide sliding window
(weights underflow beyond ~128 steps), so each 128-query tile only attends its
own key tile (diag) and the previous one (prev): O(T*256) work.

Design (v4) -- the device runs only the quadratic attention core; everything
linear in x is folded into the (cheap, O(T*C^2)) host-side shard prep:
- x^T fp16 [64, 2176] uploaded directly.
- z^T = (x @ G)^T fp16 uploaded, where G = Wq Wk^T / 8, so scores are one
  matmul per tile pair: S = x_tile^T @ z (PE, fp16, fp32 accumulate).
- V uploaded pre-scaled: vd[p, t, 0:64] = (x V)[128t+p] * e^{(p-64)*slope},
  vd[p, t, 64] = e^{(p-64)*slope} (denominator ones-column; zeroed for the
  zero-padding tile), bf16.  The remaining diag/prev ALiBi offset becomes
  constant exp biases (+20 / +20-128*slope) that cancel per-query.
- exp: bias-uniform [128, 512] ACT activations over 4-tile score batches.
- Causal mask: one DVE multiply per batch with an uploaded 0/1 mask.
- U = [P_d^T V(q+1)] + [P_p^T V(q)] accumulated in PSUM; normalize =
  reciprocal + tensor_scalar (DVE).
- Software pipeline: U/normalize/output-DMA for batch a-1 are emitted during
  batch a, so PE never stalls on the exp/mask chain.  Input DMAs are split
  across the two HWDGE rings (SP + ACT).

Sharding: 8 cores = (batch b in 0..3) x (half h in 0..1); core handles 2048
queries, receives x rows [q0-128, q0+2048) zero-padded below row 0.
"""

import numpy as np
from contextlib import ExitStack

import ml_dtypes

from concourse import bacc, mybir, tile
from concourse.bass_utils import run_bass_kernel_spmd

B, T, C, HS = 4, 4096, 64, 64
SLOPE = float((2.0**8) ** (-1.0 / 16.0))
NQ = 16               # query tiles of 128 per core
NT = NQ + 1           # key tiles per core (one extra "prev" tile below)
TLOC = NQ * 128       # 2048 queries per core
XROWS = NT * 128      # 2176 x rows per core
NCORES = 8

BIAS_D = 20.0
BIAS_P = float(20.0 - 128.0 * SLOPE)

F32 = mybir.dt.float32
F16 = mybir.dt.float16
BF16 = mybir.dt.bfloat16

_CACHE: dict = {}


def _build(loop_n=None):
    nc = bacc.Bacc("TRN2", target_bir_lowering=False, debug=False)

    xt_d = nc.dram_tensor("xt", [C, XROWS], F16, kind="ExternalInput").ap()
    zt_d = nc.dram_tensor("zt", [C, XROWS], F16, kind="ExternalInput").ap()
    vd_d = nc.dram_tensor("vd", [128, NT * 66], BF16, kind="ExternalInput").ap()
    mask_d = nc.dram_tensor("mask4", [128, 4 * 128], BF16, kind="ExternalInput").ap()
    out_d = nc.dram_tensor("out", [TLOC, HS], F32, kind="ExternalOutput").ap()

    exp_f = mybir.ActivationFunctionType.Exp

    with tile.TileContext(nc) as tc:
        with (
            tc.tile_pool(name="const", bufs=1) as cpool,
            tc.tile_pool(name="big", bufs=1) as bigp,
            tc.tile_pool(name="sdp", bufs=2, space="PSUM") as sdp,
            tc.tile_pool(name="spp", bufs=2, space="PSUM") as spp,
            tc.tile_pool(name="up", bufs=2, space="PSUM") as up,
            ExitStack() as loop_ctx,
        ):
            # --- persistent SBUF tiles (loads hoisted out of the timing loop) ---
            dummy = cpool.tile([128, 1], F32, name="dummy")
            bias_d = cpool.tile([128, 1], F32, name="bias_d")
            bias_p = cpool.tile([128, 1], F32, name="bias_p")
            mask4 = cpool.tile([128, 4, 128], BF16, name="mask4_s")
            nc.gpsimd.memset(bias_d[:], BIAS_D)
            nc.gpsimd.memset(bias_p[:], BIAS_P)
            nc.sync.dma_start(mask4[:], mask_d.rearrange("p (t c) -> p t c", c=128))

            if loop_n is not None:
                loop_ctx.enter_context(tc.For_i(0, loop_n, 1))

            xt = bigp.tile([C, XROWS], F16, name="xt_s")
            zt = bigp.tile([C, XROWS], F16, name="zt_s")
            pd = bigp.tile([128, NQ, 128], BF16, name="pd_s")
            pp = bigp.tile([128, NQ, 128], BF16, name="pp_s")
            vd = bigp.tile([128, NT * 66], BF16, name="vd_s")
            outb = bigp.tile([128, NQ, HS], F32, name="outb_s")
            recs = bigp.tile([128, NQ], F32, name="recs_s")

            # Trigger the exp table load on ACT before any real dependency.
            nc.vector.memset(dummy[:], 0.0)
            nc.scalar.activation(dummy[:], dummy[:], exp_f)

            # Input DMAs: xt/zt on the SP HWDGE ring (SP has no compute, so
            # cross-iteration WAR waits are harmless), vd on the idle
            # Pool/SWDGE ring so it overlaps the SP chain.
            half = XROWS // 2  # 1088
            nc.sync.dma_start(xt[:, 0:half], xt_d[:, 0:half])
            nc.sync.dma_start(zt[:, 0:half], zt_d[:, 0:half])
            nc.sync.dma_start(xt[:, half:XROWS], xt_d[:, half:XROWS])
            nc.sync.dma_start(zt[:, half:XROWS], zt_d[:, half:XROWS])
            nc.sync.dma_start(vd[:], vd_d)

            def u_norm_dma(b):
                # U accumulation for qtiles 4b..4b+3 (one batch behind S/exp)
                up_t = up.tile([128, 4, 65], F32, tag="u", name=f"u{b}")
                for m in range(4):
                    q = 4 * b + m
                    nc.tensor.matmul(
                        up_t[:, m, :], pd[:, q, :], vd[:, (q + 1) * 66 : (q + 1) * 66 + 65],
                        start=True, stop=False,
                    )
                    nc.tensor.matmul(
                        up_t[:, m, :], pp[:, q, :], vd[:, q * 66 : q * 66 + 65],
                        start=False, stop=True,
                    )
                nc.vector.reciprocal(recs[:, 4 * b : 4 * b + 4], up_t[:, :, 64])
                for m in range(4):
                    q = 4 * b + m
                    nc.vector.tensor_scalar_mul(
                        outb[:, q, :], up_t[:, m, 0:64], recs[:, q : q + 1]
                    )
                nc.scalar.dma_start(
                    out_d.rearrange("(n p) c -> p n c", p=128)[:, 4 * b : 4 * b + 4, :],
                    outb[:, 4 * b : 4 * b + 4, :],
                )

            for a in range(4):
                # S matmuls: diag key tiles 4a+1..4a+4, prev key tiles
                # 4a..4a+3; Sd(kt)/Sp(kt) adjacent so ldweights is shared.
                sd_t = sdp.tile([128, 4, 128], F32, tag="sd", name=f"sd{a}")
                sp_t = spp.tile([128, 4, 128], F32, tag="sp", name=f"sp{a}")
                for kt in range(4 * a, 4 * a + 5):
                    xtile = xt[:, kt * 128 : (kt + 1) * 128]
                    if kt > 4 * a:
                        # diag: queries qtile kt-1 vs key tile kt
                        nc.tensor.matmul(
                            sd_t[:, kt - 4 * a - 1, :],
                            xtile,
                            zt[:, kt * 128 : kt * 128 + 128],
                            start=True,
                            stop=True,
                        )
                    if kt < 4 * a + 4:
                        # prev: queries qtile kt vs key tile kt
                        nc.tensor.matmul(
                            sp_t[:, kt - 4 * a, :],
                            xtile,
                            zt[:, kt * 128 + 128 : kt * 128 + 256],
                            start=True,
                            stop=True,
                        )
                # exp over the 4-tile score batches (bias cancels per query)
                nc.scalar.activation(
                    pd[:, 4 * a : 4 * a + 4, :], sd_t[:], exp_f, bias=bias_d[:, 0:1]
                )
                nc.scalar.activation(
                    pp[:, 4 * a : 4 * a + 4, :], sp_t[:], exp_f, bias=bias_p[:, 0:1]
                )
                # causal mask on the 4 diag tiles: one DVE multiply
                nc.vector.tensor_mul(
                    pd[:, 4 * a : 4 * a + 4, :],
                    pd[:, 4 * a : 4 * a + 4, :],
                    mask4[:],
                )
                if a >= 1:
                    u_norm_dma(a - 1)
            u_norm_dma(3)

    nc.compile()
    return nc


def _get_nc(loop_n=None):
    key = ("nc", loop_n)
    if key not in _CACHE:
        _CACHE[key] = _build(loop_n)
    return _CACHE[key]


def make_in_maps(x, Wq, Wk, Wv):
    x = np.asarray(np.asarray(x), dtype=np.float32)
    Wq = np.asarray(np.asarray(Wq), dtype=np.float64)
    Wk = np.asarray(np.asarray(Wk), dtype=np.float64)
    Wv = np.asarray(np.asarray(Wv), dtype=np.float64)
    g = (Wq @ Wk.T * (C**-0.5)).astype(np.float32)
    pj = np.arange(128, dtype=np.float64)
    ed = np.exp((pj - 64.0) * SLOPE)
    tri = (np.arange(128)[:, None] <= np.arange(128)[None, :]).astype(
        ml_dtypes.bfloat16
    )
    mask4 = np.ascontiguousarray(np.tile(tri, (1, 4)))
    wv32 = Wv.astype(np.float32)
    in_maps = []
    for c in range(NCORES):
        b, h = divmod(c, 2)
        q0 = h * TLOC
        if h == 0:
            xs = np.concatenate(
                [np.zeros((128, C), np.float32), x[b, 0:TLOC]], axis=0
            )
        else:
            xs = x[b, q0 - 128 : q0 + TLOC]
        zs = xs @ g                       # [2176, 64] fp32
        vs = (xs @ wv32).reshape(NT, 128, HS).transpose(1, 0, 2)  # [128, 17, 64]
        vdt = np.zeros((128, NT, 66), np.float64)
        vdt[:, :, 0:64] = vs * ed[:, None, None]
        vdt[:, :, 64] = ed[:, None]
        if h == 0:
            vdt[:, 0, 64] = 0.0  # padding keys must not pollute the denominator
        in_maps.append(
            {
                "xt": np.ascontiguousarray(xs.T.astype(np.float16)),
                "zt": np.ascontiguousarray(zs.T.astype(np.float16)),
                "vd": np.ascontiguousarray(
                    vdt.reshape(128, NT * 66).astype(ml_dtypes.bfloat16)
                ),
                "mask4": mask4,
            }
        )
    return in_maps


def assemble(results):
    out = np.empty((B, T, C), dtype=np.float32)
    for c in range(NCORES):
        b, h = divmod(c, 2)
        out[b, h * TLOC : (h + 1) * TLOC] = results[c]["out"]
    return out


def run(x, Wq, Wk, Wv, trace=False, loop_n=None):
    nc = _get_nc(loop_n)
    in_maps = make_in_maps(x, Wq, Wk, Wv)
    res = run_bass_kernel_spmd(nc, in_maps, core_ids=list(range(NCORES)), trace=trace)
    return assemble(res.results), res


def kernel(x, Wq, Wk, Wv):
    out, _ = run(x, Wq, Wk, Wv, trace=False)
    return out


# revision 20
# speedup vs baseline: 1.1235x; 1.1235x over previous
"""Single-head causal attention with ALiBi (B=4, T=4096, C=HS=64) on 8 TRN2 cores.

Math: out = softmax(mask((x Wq)(x Wk)^T * C^-0.5 + (j-i)*slope)) @ (x Wv)

ALiBi slope 2^-0.5 makes the softmax an effective 256-wide sliding window
(weights underflow beyond ~128 steps), so each 128-query tile only attends its
own key tile (diag) and the previous one (prev): O(T*256) work.

Design (v4) -- the device runs only the quadratic attention core; everything
linear in x is folded into the (cheap, O(T*C^2)) host-side shard prep:
- x^T fp16 [64, 2176] uploaded directly.
- z^T = (x @ G)^T fp16 uploaded, where G = Wq Wk^T / 8, so scores are one
  matmul per tile pair: S = x_tile^T @ z (PE, fp16, fp32 accumulate).
- V uploaded pre-scaled: vd[p, t, 0:64] = (x V)[128t+p] * e^{(p-64)*slope},
  vd[p, t, 64] = e^{(p-64)*slope} (denominator ones-column; zeroed for the
  zero-padding tile), bf16.  The remaining diag/prev ALiBi offset becomes
  constant exp biases (+20 / +20-128*slope) that cancel per-query.
- exp: bias-uniform [128, 512] ACT activations over 4-tile score batches.
- Causal mask: one DVE multiply per batch with an uploaded 0/1 mask.
- U = [P_d^T V(q+1)] + [P_p^T V(q)] accumulated in PSUM; normalize =
  reciprocal + tensor_scalar (DVE).
- Software pipeline: U/normalize/output-DMA for batch a-1 are emitted during
  batch a, so PE never stalls on the exp/mask chain.  Input DMAs are split
  across the two HWDGE rings (SP + ACT).

Sharding: 8 cores = (batch b in 0..3) x (half h in 0..1); core handles 2048
queries, receives x rows [q0-128, q0+2048) zero-padded below row 0.
"""

import numpy as np
from contextlib import ExitStack

import ml_dtypes

from concourse import bacc, mybir, tile
from concourse.bass_utils import run_bass_kernel_spmd

B, T, C, HS = 4, 4096, 64, 64
SLOPE = float((2.0**8) ** (-1.0 / 16.0))
NQ = 16               # query tiles of 128 per core
NT = NQ + 1           # key tiles per core (one extra "prev" tile below)
TLOC = NQ * 128       # 2048 queries per core
XROWS = NT * 128      # 2176 x rows per core
NCORES = 8

BIAS_D = 20.0
BIAS_P = float(20.0 - 128.0 * SLOPE)

F32 = mybir.dt.float32
F16 = mybir.dt.float16
BF16 = mybir.dt.bfloat16

_CACHE: dict = {}


def _build(loop_n=None):
    nc = bacc.Bacc("TRN2", target_bir_lowering=False, debug=False)

    xt_d = nc.dram_tensor("xt", [C, XROWS], F16, kind="ExternalInput").ap()
    zt_d = nc.dram_tensor("zt", [C, XROWS], F16, kind="ExternalInput").ap()
    vd_d = nc.dram_tensor("vd", [128, NT * 66], BF16, kind="ExternalInput").ap()
    mask_d = nc.dram_tensor("mask4", [128, 4 * 128], BF16, kind="ExternalInput").ap()
    out_d = nc.dram_tensor("out", [TLOC, HS], F32, kind="ExternalOutput").ap()

    exp_f = mybir.ActivationFunctionType.Exp

    with tile.TileContext(nc) as tc:
        with (
            tc.tile_pool(name="const", bufs=1) as cpool,
            tc.tile_pool(name="big", bufs=1) as bigp,
            tc.tile_pool(name="sdp", bufs=2, space="PSUM") as sdp,
            tc.tile_pool(name="spp", bufs=2, space="PSUM") as spp,
            tc.tile_pool(name="up", bufs=2, space="PSUM") as up,
            ExitStack() as loop_ctx,
        ):
            # --- persistent SBUF tiles (loads hoisted out of the timing loop) ---
            dummy = cpool.tile([128, 1], F32, name="dummy")
            bias_d = cpool.tile([128, 1], F32, name="bias_d")
            bias_p = cpool.tile([128, 1], F32, name="bias_p")
            mask4 = cpool.tile([128, 4, 128], BF16, name="mask4_s")
            nc.gpsimd.memset(bias_d[:], BIAS_D)
            nc.gpsimd.memset(bias_p[:], BIAS_P)
            nc.sync.dma_start(mask4[:], mask_d.rearrange("p (t c) -> p t c", c=128))

            if loop_n is not None:
                loop_ctx.enter_context(tc.For_i(0, loop_n, 1))

            xt = bigp.tile([C, XROWS], F16, name="xt_s")
            zt = bigp.tile([C, XROWS], F16, name="zt_s")
            pd = bigp.tile([128, NQ, 128], BF16, name="pd_s")
            pp = bigp.tile([128, NQ, 128], BF16, name="pp_s")
            vd = bigp.tile([128, NT * 66], BF16, name="vd_s")
            outb = bigp.tile([128, NQ, HS], F32, name="outb_s")
            recs = bigp.tile([128, NQ], F32, name="recs_s")

            # Trigger the exp table load on ACT before any real dependency.
            nc.vector.memset(dummy[:], 0.0)
            nc.scalar.activation(dummy[:], dummy[:], exp_f)

            # Input DMAs: xt/zt on the SP HWDGE ring (SP has no compute, so
            # cross-iteration WAR waits are harmless), vd on the idle
            # Pool/SWDGE ring so it overlaps the SP chain.
            half = XROWS // 2  # 1088
            nc.sync.dma_start(xt[:, 0:half], xt_d[:, 0:half])
            nc.sync.dma_start(zt[:, 0:half], zt_d[:, 0:half])
            nc.sync.dma_start(xt[:, half:XROWS], xt_d[:, half:XROWS])
            nc.sync.dma_start(zt[:, half:XROWS], zt_d[:, half:XROWS])
            nc.sync.dma_start(vd[:], vd_d)

            def u_norm_dma(b):
                # U accumulation for qtiles 4b..4b+3 (one batch behind S/exp)
                up_t = up.tile([128, 4, 65], F32, tag="u", name=f"u{b}")
                for m in range(4):
                    q = 4 * b + m
                    nc.tensor.matmul(
                        up_t[:, m, :], pd[:, q, :], vd[:, (q + 1) * 66 : (q + 1) * 66 + 65],
                        start=True, stop=False,
                    )
                    nc.tensor.matmul(
                        up_t[:, m, :], pp[:, q, :], vd[:, q * 66 : q * 66 + 65],
                        start=False, stop=True,
                    )
                nc.vector.reciprocal(recs[:, 4 * b : 4 * b + 4], up_t[:, :, 64])
                for m in range(4):
                    q = 4 * b + m
                    nc.vector.tensor_scalar_mul(
                        outb[:, q, :], up_t[:, m, 0:64], recs[:, q : q + 1]
                    )
                nc.sync.dma_start(
                    out_d.rearrange("(n p) c -> p n c", p=128)[:, 4 * b : 4 * b + 4, :],
                    outb[:, 4 * b : 4 * b + 4, :],
                )

            for a in range(4):
                # S matmuls: diag key tiles 4a+1..4a+4, prev key tiles
                # 4a..4a+3; Sd(kt)/Sp(kt) adjacent so ldweights is shared.
                sd_t = sdp.tile([128, 4, 128], F32, tag="sd", name=f"sd{a}")
                sp_t = spp.tile([128, 4, 128], F32, tag="sp", name=f"sp{a}")
                for kt in range(4 * a, 4 * a + 5):
                    xtile = xt[:, kt * 128 : (kt + 1) * 128]
                    if kt > 4 * a:
                        # diag: queries qtile kt-1 vs key tile kt
                        nc.tensor.matmul(
                            sd_t[:, kt - 4 * a - 1, :],
                            xtile,
                            zt[:, kt * 128 : kt * 128 + 128],
                            start=True,
                            stop=True,
                        )
                    if kt < 4 * a + 4:
                        # prev: queries qtile kt vs key tile kt
                        nc.tensor.matmul(
                            sp_t[:, kt - 4 * a, :],
                            xtile,
                            zt[:, kt * 128 + 128 : kt * 128 + 256],
                            start=True,
                            stop=True,
                        )
                # exp over the 4-tile score batches (bias cancels per query)
                nc.scalar.activation(
                    pd[:, 4 * a : 4 * a + 4, :], sd_t[:], exp_f, bias=bias_d[:, 0:1]
                )
                nc.scalar.activation(
                    pp[:, 4 * a : 4 * a + 4, :], sp_t[:], exp_f, bias=bias_p[:, 0:1]
                )
                # causal mask on the 4 diag tiles: one DVE multiply
                nc.vector.tensor_mul(
                    pd[:, 4 * a : 4 * a + 4, :],
                    pd[:, 4 * a : 4 * a + 4, :],
                    mask4[:],
                )
                if a >= 1:
                    u_norm_dma(a - 1)
            u_norm_dma(3)

    nc.compile()
    return nc


def _get_nc(loop_n=None):
    key = ("nc", loop_n)
    if key not in _CACHE:
        _CACHE[key] = _build(loop_n)
    return _CACHE[key]


def make_in_maps(x, Wq, Wk, Wv):
    x = np.asarray(np.asarray(x), dtype=np.float32)
    Wq = np.asarray(np.asarray(Wq), dtype=np.float64)
    Wk = np.asarray(np.asarray(Wk), dtype=np.float64)
    Wv = np.asarray(np.asarray(Wv), dtype=np.float64)
    g = (Wq @ Wk.T * (C**-0.5)).astype(np.float32)
    pj = np.arange(128, dtype=np.float64)
    ed = np.exp((pj - 64.0) * SLOPE)
    tri = (np.arange(128)[:, None] <= np.arange(128)[None, :]).astype(
        ml_dtypes.bfloat16
    )
    mask4 = np.ascontiguousarray(np.tile(tri, (1, 4)))
    wv32 = Wv.astype(np.float32)
    in_maps = []
    for c in range(NCORES):
        b, h = divmod(c, 2)
        q0 = h * TLOC
        if h == 0:
            xs = np.concatenate(
                [np.zeros((128, C), np.float32), x[b, 0:TLOC]], axis=0
            )
        else:
            xs = x[b, q0 - 128 : q0 + TLOC]
        zs = xs @ g                       # [2176, 64] fp32
        vs = (xs @ wv32).reshape(NT, 128, HS).transpose(1, 0, 2)  # [128, 17, 64]
        vdt = np.zeros((128, NT, 66), np.float64)
        vdt[:, :, 0:64] = vs * ed[:, None, None]
        vdt[:, :, 64] = ed[:, None]
        if h == 0:
            vdt[:, 0, 64] = 0.0  # padding keys must not pollute the denominator
        in_maps.append(
            {
                "xt": np.ascontiguousarray(xs.T.astype(np.float16)),
                "zt": np.ascontiguousarray(zs.T.astype(np.float16)),
                "vd": np.ascontiguousarray(
                    vdt.reshape(128, NT * 66).astype(ml_dtypes.bfloat16)
                ),
                "mask4": mask4,
            }
        )
    return in_maps


def assemble(results):
    out = np.empty((B, T, C), dtype=np.float32)
    for c in range(NCORES):
        b, h = divmod(c, 2)
        out[b, h * TLOC : (h + 1) * TLOC] = results[c]["out"]
    return out


def run(x, Wq, Wk, Wv, trace=False, loop_n=None):
    nc = _get_nc(loop_n)
    in_maps = make_in_maps(x, Wq, Wk, Wv)
    res = run_bass_kernel_spmd(nc, in_maps, core_ids=list(range(NCORES)), trace=trace)
    return assemble(res.results), res


def kernel(x, Wq, Wk, Wv):
    out, _ = run(x, Wq, Wk, Wv, trace=False)
    return out
